# revision 15
# baseline (speedup 1.0000x reference)
"""Trainium2 Bass kernel for a 2-layer LSTM encoder/decoder forecaster.

Model (per batch element):
  teacher-forced over S=168 steps:  enc -> LSTM0 -> LSTM1 (keep last out)
  autoregressive rollout for 23 more steps feeding decoder output back.

Sharding: data-parallel, batch 1024 -> 8 cores x 128. All weights are
replicated and SBUF-resident; zero inter-core communication.

Layout: everything is FEATURE-MAJOR. Gates are computed as 16 chunks of
[128 gate-rows x 128 batch] PSUM tiles, with the (transposed, chunked)
weights as the stationary matmul operand and h / x as the moving
operand. Because the cell elementwise output h = sig(o)*tanh(c) is then
produced directly in [feature, batch] layout, it is ALREADY the k-chunk
operand the next step's recurrent matmuls need - no PE transposes, no
PSUM->SBUF copies of state anywhere in the loop.

Everything is bf16 on the matmul path (full PE rate at any width, and a
validated ~3e-3 end-to-end error vs the 2e-2 budget; fp8 was measured at
4.3e-2 and rejected). The cell state c stays fp32.

Gate chunk order after host-side row permutation: (f, i, o, g) so that
GA = [f|i] (chunks 0..7) and GB = [o|g] (chunks 8..15), letting each
activation read one contiguous PSUM span.

The encoder is algebraically fused into layer 0 (M0 = Wih0 @ W_enc, bias
folded into an appended ones-row of the feature-major input). Layer 1's
bias enters via 4 ROW-TILED selector matmuls (K=4 each, tile_position
(32j, 0)) that run CONCURRENTLY in the four 32-row groups of the PE
array - one ~512-cycle span instead of four (saves ~780ns/step vs the
sequential version). The bias/selector operands are replicated at
partitions 32j..32j+3 so each row group streams its own data. The
decoder is augmented with a column that regenerates the ones-row so the
AR feedback tile needs no fixup at all: the decoder's SBUF output IS the
next step's input operand.

PE order per TF step: [xa(t), whh0(t), bias1(t)] prefetched at the end
of step t-1, then whh1(t), wih1(t) - every matmul's input is ready
before PE reaches it, keeping PE (the bottleneck) gapless.

AR steps reorder the tail to hide the feedback serialization: after
wih1(t), PE runs [bias1(t+1) ga-banks, whh0(t+1)] - 4.5us of work that
needs only h0(t) - while cell1(t)'s ACT/DVE chain produces h1(t). Only
then come the decoder, the dout bias-add (on DVE - ACT's FIFO latency
would sit on the critical path), the xa(t+1) feedback matmuls (which
accumulate INTO the already-written whh0 PSUM and are on the critical
path to cell0(t+1)), and last the gb1 bias banks. This keeps the PE
idle gap per AR step under ~0.3us where the previous ordering idled
4-5us per step (long enough for the HAM clock-gate to re-throttle the
PE to half clock). dec_ps aliases the gb1 PSUM slot (consumed by
cell1's ACT reads by then); the gb1 bias banks are re-zeroed after the
dout read.

Two latency tricks keep the h-consuming matmuls fed: emit_rec walks
k-outer within each 8-chunk half (first 8 matmuls need only h chunk 0,
matching the cell's staggered per-chunk h production), and the cell
splits tanh_g / ig / c-add / tanh_c into 2-chunk halves so h chunk 0
emerges ~0.7us sooner. Measured: TF steady state 12.05us/step (the
bf16 N=128 issue-rate wall: 208 matmuls x ~56ns + one ~470ns bias
span), AR 12.80us/step.
"""

import sys
import threading

sys.path.insert(0, "/opt/trn_rl_repo")

import numpy as np
import ml_dtypes

PRED_LEN = 24
F, I, H = 64, 128, 512
B, S = 1024, 168
NCORES = 8
BL = B // NCORES          # batch per core = 128
G = 4 * H                 # gate width 2048
NCH = G // 128            # 16 gate chunks
KCH = H // 128            # 4 k-chunks of the hidden dim
KX = F + 1                # x operand rows incl. ones row = 65
FD = F + 2                # decoder rows: 64 outputs + ones + pad = 66

BF16NP = ml_dtypes.bfloat16

_cache = {}
_cache_lock = threading.Lock()


def _gate_perm():
    # pytorch gate order i,f,g,o -> reorder rows to (f,i,o,g): chunks
    # 0-3=f, 4-7=i (-> GA), 8-11=o, 12-15=g (-> GB).
    return np.concatenate([
        np.arange(H, 2 * H),        # f
        np.arange(0, H),            # i
        np.arange(3 * H, 4 * H),    # o
        np.arange(2 * H, 3 * H),    # g
    ])


def _build_program(n_tf=S, n_ar=PRED_LEN - 1):
    import concourse.bacc as bacc
    import concourse.tile as tile
    import concourse.mybir as mybir

    F32 = mybir.dt.float32
    BF16 = mybir.dt.bfloat16
    AF = mybir.ActivationFunctionType

    nc = bacc.Bacc("TRN2", target_bir_lowering=False, debug=False,
                   num_devices=NCORES)

    x_d = nc.dram_tensor("xT", [KX, n_tf, BL], BF16, kind="ExternalInput").ap()
    m0_d = nc.dram_tensor("m0t", [KX, G], BF16, kind="ExternalInput").ap()
    whh0_d = nc.dram_tensor("whh0t", [128, KCH, G], BF16, kind="ExternalInput").ap()
    wih1_d = nc.dram_tensor("wih1t", [128, KCH, G], BF16, kind="ExternalInput").ap()
    whh1_d = nc.dram_tensor("whh1t", [128, KCH, G], BF16, kind="ExternalInput").ap()
    b1r_d = nc.dram_tensor("b1r", [128, 128], BF16, kind="ExternalInput").ap()
    selr_d = nc.dram_tensor("selr", [128, 512], BF16, kind="ExternalInput").ap()
    wdec_d = nc.dram_tensor("wdect", [128, KCH, FD], BF16, kind="ExternalInput").ap()
    bdec_d = nc.dram_tensor("bdec", [FD, 1], F32, kind="ExternalInput").ap()
    y_d = nc.dram_tensor("y", [n_ar + 1, F, BL], BF16, kind="ExternalOutput").ap()

    from contextlib import ExitStack
    with tile.TileContext(nc) as tc, ExitStack() as ctx:
        wpool = ctx.enter_context(tc.tile_pool(name="w", bufs=1))
        spool = ctx.enter_context(tc.tile_pool(name="s", bufs=2))
        hpool = ctx.enter_context(tc.tile_pool(name="h", bufs=2))
        dpool = ctx.enter_context(tc.tile_pool(name="d", bufs=2))
        gpool = ctx.enter_context(tc.tile_pool(name="g", bufs=1, space="PSUM"))

        # ---- resident weights + input ----
        # DMA order matters: everything step 0 touches (x head, m0, b1,
        # ones) loads FIRST; the three 2MB weight matrices follow in
        # first-use order; the 2.7MB x tail (not needed until step 8)
        # goes last. This cut a measured ~25us lead-in stall where
        # bias1(0)/wih1(0) sat behind the bulk transfers.
        x_sb = wpool.tile([KX, n_tf, BL], BF16)
        x_head = min(8, n_tf)
        nc.sync.dma_start(x_sb[:, 0:x_head, :], x_d[:, 0:x_head, :])
        m0_sb = wpool.tile([KX, G], BF16)
        nc.sync.dma_start(m0_sb[:], m0_d[:])
        b1r_sb = wpool.tile([128, 128], BF16)
        nc.scalar.dma_start(b1r_sb[:], b1r_d[:])
        selr_sb = wpool.tile([128, 512], BF16)
        nc.scalar.dma_start(selr_sb[:], selr_d[:])
        # The big weight matrices arrive as per-k-chunk DMAs on the SYNC
        # hardware queue in first-use order: wih1 (step 0 mid), whh0
        # (step 0 tail), whh1 (step 1); emit_rec walks k-outer within
        # each half, matching this arrival order. All bulk data stays on
        # sync: DMA triggers on the scalar/Act queue measurably BLOCK the
        # ACT engine's instruction FIFO until the transfer completes
        # (sig_fi(0) stalled 9us when weights rode that queue), and
        # gpsimd's software DGE is ~3x slower to deliver. Only the two
        # tiny bias/selector tiles ride the scalar queue (they land ~5us,
        # well before the first ACT op at ~15us).
        wih1_sb = wpool.tile([128, KCH, G], BF16)
        whh0_sb = wpool.tile([128, KCH, G], BF16)
        whh1_sb = wpool.tile([128, KCH, G], BF16)
        for w_sb, w_d in ((wih1_sb, wih1_d), (whh0_sb, whh0_d),
                          (whh1_sb, whh1_d)):
            for k in range(KCH):
                nc.sync.dma_start(w_sb[:, k], w_d[:, k])
        wdec_sb = wpool.tile([128, KCH, FD], BF16)
        nc.sync.dma_start(wdec_sb[:], wdec_d[:])
        bdec_sb = wpool.tile([FD, 1], F32)
        nc.sync.dma_start(bdec_sb[:], bdec_d[:])
        if x_head < n_tf:
            nc.sync.dma_start(x_sb[:, x_head:, :], x_d[:, x_head:, :])

        def halves(ga, gb, m):
            return (ga if m < 8 else gb)[:, m % 8, :]

        # start=True lazily zeroes the WHOLE 2KB PSUM bank (4 of our 512B
        # chunk regions), so only the first write per bank may set it; the
        # sibling regions still see the bank's pending-zero and overwrite.
        def emit_xa(ga, gb, rhs, only=False, accum=False):
            # input-side gate contribution. `only`=True closes the group
            # (t=0 has no recurrent term). `accum`=True: the recurrent
            # matmuls already wrote (and zeroed) the banks, so accumulate
            # on top and close each chunk's group.
            for m in range(NCH):
                nc.tensor.matmul(halves(ga, gb, m),
                                 m0_sb[:, m * 128:(m + 1) * 128], rhs,
                                 start=(not accum) and (m % 4 == 0),
                                 stop=only or accum,
                                 skip_group_check=True)

        def emit_bias1(ga, gb, banks=(0, 1, 2, 3)):
            # One K=4 selector matmul per PSUM bank, row-TILED into the
            # four 32-row groups of the PE array so consecutive banks run
            # concurrently (span ~= one N=512 matmul, not four).
            # out[p, c*128+b] = sum_k b1r[32j+k, p] * selr[32j+k, c*128+b]
            #                 = b1[(4j+c)*128 + p].
            for j in banks:
                gt = ga if j < 2 else gb
                out = gt[:, (j % 2) * 4:(j % 2) * 4 + 4, :]
                nc.tensor.matmul(out, b1r_sb[32 * j:32 * j + 4, :],
                                 selr_sb[32 * j:32 * j + 4, :],
                                 start=True, stop=False,
                                 skip_group_check=True,
                                 tile_position=(32 * j, 0))

        def emit_rec(w_sb, h, ga, gb, last, zero_start=False):
            # k-outer within each 8-chunk half: the first 8 matmuls need
            # only h chunk 0, the next 8 only chunk 1, ... so PE never
            # waits for the cell's staggered per-chunk h production (the
            # DVE emits chunks ~210ns apart; m-outer would need all four
            # within 230ns). The PSUM completion position of each gate
            # half is unchanged (still matmul #32 / #64), so the
            # downstream activation timing is identical.
            for half in range(2):
                ms = range(half * 8, half * 8 + 8)
                for k in range(KCH):
                    for m in ms:
                        nc.tensor.matmul(halves(ga, gb, m),
                                         w_sb[:, k, m * 128:(m + 1) * 128],
                                         h[:, k, :],
                                         start=(zero_start and k == 0
                                                and m % 4 == 0),
                                         stop=(last and k == KCH - 1),
                                         skip_group_check=True)

        def cell(ga, gb, c_prev, l):
            # The serial chain gates -> h bounds how early the next step's
            # h-consuming matmuls can run (the PE covers it with ~3.6us of
            # h-independent work in AR steps). tanh_g / ig / add / tanh_c
            # are split into 2-chunk halves so h chunk 0 emerges ~0.7us
            # sooner; the halves pipeline across the ACT/DVE queues.
            sig_fi = spool.tile([128, 8, BL], F32, tag=f"sfi{l}")
            nc.scalar.activation(sig_fi[:], ga[:], AF.Sigmoid)
            tanh_g = spool.tile([128, KCH, BL], F32, tag=f"tg{l}")
            nc.scalar.activation(tanh_g[:, 0:2, :], gb[:, 4:6, :], AF.Tanh)
            nc.scalar.activation(tanh_g[:, 2:4, :], gb[:, 6:8, :], AF.Tanh)
            sig_o = spool.tile([128, KCH, BL], F32, tag=f"so{l}")
            nc.scalar.activation(sig_o[:], gb[:, 0:4, :], AF.Sigmoid)
            # fc first on the DVE queue: its input sig_fi is ready one Act
            # op earlier than ig's tanh_g, shortening the serial cell chain.
            c_new = hpool.tile([128, KCH, BL], F32, tag=f"c{l}")
            ig = spool.tile([128, KCH, BL], F32, tag=f"ig{l}")
            tanh_c = spool.tile([128, KCH, BL], F32, tag=f"tc{l}")
            if c_prev is None:
                for a, b in ((0, 2), (2, 4)):
                    nc.vector.tensor_mul(ig[:, a:b, :],
                                         sig_fi[:, 4 + a:4 + b, :],
                                         tanh_g[:, a:b, :])
                    nc.vector.tensor_copy(c_new[:, a:b, :], ig[:, a:b, :])
                    nc.scalar.activation(tanh_c[:, a:b, :], c_new[:, a:b, :],
                                         AF.Tanh)
            else:
                fc = spool.tile([128, KCH, BL], F32, tag=f"fc{l}")
                nc.vector.tensor_mul(fc[:], sig_fi[:, 0:4, :], c_prev[:])
                for a, b in ((0, 2), (2, 4)):
                    nc.vector.tensor_mul(ig[:, a:b, :],
                                         sig_fi[:, 4 + a:4 + b, :],
                                         tanh_g[:, a:b, :])
                    nc.vector.tensor_add(c_new[:, a:b, :], fc[:, a:b, :],
                                         ig[:, a:b, :])
                    nc.scalar.activation(tanh_c[:, a:b, :], c_new[:, a:b, :],
                                         AF.Tanh)
            h_new = hpool.tile([128, KCH, BL], BF16, tag=f"h{l}")
            for k in range(KCH):
                nc.vector.tensor_mul(h_new[:, k, :], sig_o[:, k, :],
                                     tanh_c[:, k, :])
            return c_new, h_new

        def emit_dec(j, h1):
            # dec_ps aliases the gb1 slot: cell1(t)'s ACT reads of gb1 are
            # long done when PE reaches the decoder. The gb1 bias banks
            # for step t+1 are (re-)zeroed AFTER the dout read.
            dec_ps = gpool.tile([FD, BL], F32, tag="gb1")
            for k in range(KCH):
                nc.tensor.matmul(dec_ps[:], wdec_sb[:, k, :], h1[:, k, :],
                                 start=(k == 0), stop=(k == KCH - 1))
            # Bias-add on DVE, not ACT: DVE is idle here (h1 muls just
            # drained) while ACT's FIFO latency would sit on the critical
            # dec -> dout -> xa feedback path.
            dout = dpool.tile([FD, BL], BF16, tag="dout")
            nc.vector.tensor_scalar_add(dout[:], dec_ps[:], bdec_sb[:])
            nc.sync.dma_start(y_d[j], dout[0:F, :])
            return dout

        n_steps = n_tf + n_ar
        h0 = h1 = c0 = c1 = None
        ga0 = gb0 = ga1 = gb1 = None
        for t in range(n_steps):
            if t == 0:
                ga0 = gpool.tile([128, 8, BL], F32, tag="ga0")
                gb0 = gpool.tile([128, 8, BL], F32, tag="gb0")
                emit_xa(ga0, gb0, x_sb[:, 0, :], only=True)
            c0, h0 = cell(ga0, gb0, c0, 0)
            if t == 0:
                ga1 = gpool.tile([128, 8, BL], F32, tag="ga1")
                gb1 = gpool.tile([128, 8, BL], F32, tag="gb1")
                emit_bias1(ga1, gb1)
            else:
                emit_rec(whh1_sb, h1, ga1, gb1, last=False)
            emit_rec(wih1_sb, h0, ga1, gb1, last=True)
            c1, h1 = cell(ga1, gb1, c1, 1)

            next_is_ar = n_tf <= t + 1 < n_steps
            if next_is_ar:
                # Fill the wait for h1(t) with h1-independent work for
                # step t+1: the ga1 bias banks and ALL of whh0(t+1)
                # (~4.5us of PE work vs the ~4us cell1 latency).
                ga1n = gpool.tile([128, 8, BL], F32, tag="ga1")
                emit_bias1(ga1n, None, banks=(0, 1))
                ga0 = gpool.tile([128, 8, BL], F32, tag="ga0")
                gb0 = gpool.tile([128, 8, BL], F32, tag="gb0")
                emit_rec(whh0_sb, h0, ga0, gb0, last=False, zero_start=True)
                dout = emit_dec(t - (n_tf - 1), h1)
                # xa BEFORE the gb1 bias banks: both wait on dout, but xa
                # is on the critical path to cell0(t+1) -> h0 -> wih1(t+1)
                # while bias23 only needs to precede whh1(t+1).
                emit_xa(ga0, gb0, dout[0:KX, :], accum=True)
                gb1n = gpool.tile([128, 8, BL], F32, tag="gb1")
                emit_bias1(None, gb1n, banks=(2, 3))
                ga1, gb1 = ga1n, gb1n
            elif t + 1 < n_steps:
                # TF next step: x is SBUF-resident, no feedback hazard.
                ga0 = gpool.tile([128, 8, BL], F32, tag="ga0")
                gb0 = gpool.tile([128, 8, BL], F32, tag="gb0")
                emit_xa(ga0, gb0, x_sb[:, t + 1, :])
                emit_rec(whh0_sb, h0, ga0, gb0, last=True)
                ga1 = gpool.tile([128, 8, BL], F32, tag="ga1")
                gb1 = gpool.tile([128, 8, BL], F32, tag="gb1")
                emit_bias1(ga1, gb1)
            else:
                emit_dec(t - (n_tf - 1), h1)

    nc.compile()
    return nc


def _get_program(n_tf=S, n_ar=PRED_LEN - 1):
    key = (n_tf, n_ar)
    with _cache_lock:
        if key not in _cache:
            _cache[key] = _build_program(n_tf, n_ar)
        return _cache[key]


def _kmajor(w):
    """[H, N] -> [128, KCH, N]: row h = k*128 + p lands at [p, k, :]."""
    n = w.shape[1]
    return np.ascontiguousarray(
        w.reshape(KCH, 128, n).transpose(1, 0, 2)).astype(BF16NP)


def _prep_weights(W_enc, b_enc, Wih0, Whh0, bih0, bhh0,
                  Wih1, Whh1, bih1, bhh1, W_dec, b_dec):
    perm = _gate_perm()
    f32 = np.float32

    M0 = (Wih0 @ W_enc)[perm]                                   # [G, F]
    b0 = (Wih0 @ b_enc + bih0 + bhh0)[perm]                     # [G]
    m0t = np.concatenate([M0.T, b0[None, :]], axis=0)           # [KX, G]

    b1p = (bih1 + bhh1)[perm]                                   # [G]
    # Row-tiled bias layout: bank j's K=4 selector operands live at
    # partitions 32j..32j+3 (PE row group j).
    b1r = np.zeros((128, 128), f32)
    selr = np.zeros((128, 512), f32)
    for j in range(KCH):
        for k in range(KCH):
            b1r[32 * j + k, :] = b1p[(4 * j + k) * 128:(4 * j + k + 1) * 128]
            selr[32 * j + k, k * 128:(k + 1) * 128] = 1.0

    wdec_aug = np.concatenate(
        [W_dec.T, np.zeros((H, 2), f32)], axis=1)               # [H, FD]
    bdec = np.concatenate([b_dec, np.ones((1,), f32), np.zeros((1,), f32)])

    return {
        "m0t": np.ascontiguousarray(m0t).astype(BF16NP),
        "whh0t": _kmajor(np.ascontiguousarray(Whh0[perm].T)),
        "wih1t": _kmajor(np.ascontiguousarray(Wih1[perm].T)),
        "whh1t": _kmajor(np.ascontiguousarray(Whh1[perm].T)),
        "b1r": b1r.astype(BF16NP),
        "selr": selr.astype(BF16NP),
        "wdect": _kmajor(wdec_aug),
        "bdec": np.ascontiguousarray(bdec[:, None], f32),
    }


def _make_in_maps(x, weights, _n_tf=S):
    in_maps = []
    for c in range(NCORES):
        xs = x[c * BL:(c + 1) * BL, :_n_tf, :]                # [BL, n_tf, F]
        xT = xs.transpose(2, 1, 0)                            # [F, n_tf, BL]
        xa = np.concatenate(
            [xT, np.ones((1, _n_tf, BL), np.float32)], axis=0)  # [KX, n_tf, BL]
        in_maps.append(
            {"xT": np.ascontiguousarray(xa).astype(BF16NP), **weights})
    return in_maps


def kernel(x, W_enc, b_enc, Wih0, Whh0, bih0, bhh0,
           Wih1, Whh1, bih1, bhh1, W_dec, b_dec, _n_tf=S, _n_ar=PRED_LEN - 1):
    from concourse.bass_utils import run_bass_kernel_spmd

    x = np.asarray(x, np.float32)
    weights = _prep_weights(
        np.asarray(W_enc, np.float32), np.asarray(b_enc, np.float32),
        np.asarray(Wih0, np.float32), np.asarray(Whh0, np.float32),
        np.asarray(bih0, np.float32), np.asarray(bhh0, np.float32),
        np.asarray(Wih1, np.float32), np.asarray(Whh1, np.float32),
        np.asarray(bih1, np.float32), np.asarray(bhh1, np.float32),
        np.asarray(W_dec, np.float32), np.asarray(b_dec, np.float32))

    nc = _get_program(_n_tf, _n_ar)
    in_maps = _make_in_maps(x, weights, _n_tf)
    res = run_bass_kernel_spmd(nc, in_maps, core_ids=list(range(NCORES)))

    out = np.empty((B, _n_ar + 1, F), np.float32)
    for c in range(NCORES):
        y = np.asarray(res.results[c]["y"], dtype=np.float32)  # [n_out, F, BL]
        out[c * BL:(c + 1) * BL] = y.transpose(2, 0, 1)
    return out


# revision 18
# speedup vs baseline: 1.1981x; 1.1981x over previous
"""Trainium2 Bass kernel for a 2-layer LSTM encoder/decoder forecaster.

Model (per batch element):
  teacher-forced over S=168 steps:  enc -> LSTM0 -> LSTM1 (keep last out)
  autoregressive rollout for 23 more steps feeding decoder output back.

Sharding: data-parallel, batch 1024 -> 8 cores x 128. All weights are
replicated and SBUF-resident; zero inter-core communication.

Layout: everything is FEATURE-MAJOR. Gates are computed as 16 chunks of
[128 gate-rows x 128 batch] PSUM tiles, with the (transposed, chunked)
weights as the stationary matmul operand and h / x as the moving
operand. Because the cell elementwise output h = sig(o)*tanh(c) is then
produced directly in [feature, batch] layout, it is ALREADY the k-chunk
operand the next step's recurrent matmuls need - no PE transposes, no
PSUM->SBUF copies of state anywhere in the loop.

Everything is bf16 on the matmul path (full PE rate at any width, and a
validated ~3e-3 end-to-end error vs the 2e-2 budget; fp8 was measured at
4.3e-2 and rejected). The cell state c stays fp32.

Gate chunk order after host-side row permutation: (f, i, o, g) so that
GA = [f|i] (chunks 0..7) and GB = [o|g] (chunks 8..15), letting each
activation read one contiguous PSUM span.

The encoder is algebraically fused into layer 0 (M0 = Wih0 @ W_enc, bias
folded into an appended ones-row of the feature-major input). Layer 1's
bias enters via 4 ROW-TILED selector matmuls (K=4 each, tile_position
(32j, 0)) that run CONCURRENTLY in the four 32-row groups of the PE
array - one ~512-cycle span instead of four (saves ~780ns/step vs the
sequential version). The bias/selector operands are replicated at
partitions 32j..32j+3 so each row group streams its own data. The
decoder is augmented with a column that regenerates the ones-row so the
AR feedback tile needs no fixup at all: the decoder's SBUF output IS the
next step's input operand.

PE order per TF step: [xa(t), whh0(t), bias1(t)] prefetched at the end
of step t-1, then whh1(t), wih1(t) - every matmul's input is ready
before PE reaches it, keeping PE (the bottleneck) gapless.

AR steps reorder the tail to hide the feedback serialization: after
wih1(t), PE runs [bias1(t+1) ga-banks, whh0(t+1)] - 4.5us of work that
needs only h0(t) - while cell1(t)'s ACT/DVE chain produces h1(t). Only
then come the decoder, the dout bias-add (on DVE - ACT's FIFO latency
would sit on the critical path), the xa(t+1) feedback matmuls (which
accumulate INTO the already-written whh0 PSUM and are on the critical
path to cell0(t+1)), and last the gb1 bias banks. This keeps the PE
idle gap per AR step under ~0.3us where the previous ordering idled
4-5us per step (long enough for the HAM clock-gate to re-throttle the
PE to half clock). dec_ps aliases the gb1 PSUM slot (consumed by
cell1's ACT reads by then); the gb1 bias banks are re-zeroed after the
dout read.

Two latency tricks keep the h-consuming matmuls fed: emit_rec walks
k-outer within each 8-chunk half (first 8 matmuls need only h chunk 0,
matching the cell's staggered per-chunk h production), and the cell
splits tanh_g / ig / c-add / tanh_c into 2-chunk halves so h chunk 0
emerges ~0.7us sooner. Measured: TF steady state 12.05us/step (the
bf16 N=128 issue-rate wall: 208 matmuls x ~56ns + one ~470ns bias
span), AR 12.80us/step.
"""

import sys
import threading

sys.path.insert(0, "/opt/trn_rl_repo")

import numpy as np
import ml_dtypes

PRED_LEN = 24
F, I, H = 64, 128, 512
B, S = 1024, 168
NCORES = 8
BL = B // NCORES          # batch per core = 128
G = 4 * H                 # gate width 2048
NCH = G // 128            # 16 gate chunks
KCH = H // 128            # 4 k-chunks of the hidden dim
KX = F + 1                # x operand rows incl. ones row = 65
FD = F + 2                # decoder rows: 64 outputs + ones + pad = 66

BF16NP = ml_dtypes.bfloat16

_cache = {}
_cache_lock = threading.Lock()


def _gate_perm():
    # pytorch gate order i,f,g,o -> reorder rows to (f,i,o,g): chunks
    # 0-3=f, 4-7=i (-> GA), 8-11=o, 12-15=g (-> GB).
    return np.concatenate([
        np.arange(H, 2 * H),        # f
        np.arange(0, H),            # i
        np.arange(3 * H, 4 * H),    # o
        np.arange(2 * H, 3 * H),    # g
    ])


def _build_program(n_tf=S, n_ar=PRED_LEN - 1):
    import concourse.bacc as bacc
    import concourse.tile as tile
    import concourse.mybir as mybir

    F32 = mybir.dt.float32
    BF16 = mybir.dt.bfloat16
    AF = mybir.ActivationFunctionType

    nc = bacc.Bacc("TRN2", target_bir_lowering=False, debug=False,
                   num_devices=NCORES)

    x_d = nc.dram_tensor("xT", [KX, n_tf, BL], BF16, kind="ExternalInput").ap()
    m0_d = nc.dram_tensor("m0t", [KX, G], BF16, kind="ExternalInput").ap()
    whh0_d = nc.dram_tensor("whh0t", [128, KCH, G], BF16, kind="ExternalInput").ap()
    wih1_d = nc.dram_tensor("wih1t", [128, KCH, G], BF16, kind="ExternalInput").ap()
    whh1_d = nc.dram_tensor("whh1t", [128, KCH, G], BF16, kind="ExternalInput").ap()
    b1r_d = nc.dram_tensor("b1r", [128, 128], BF16, kind="ExternalInput").ap()
    selr_d = nc.dram_tensor("selr", [128, 512], BF16, kind="ExternalInput").ap()
    wdec_d = nc.dram_tensor("wdect", [128, KCH, FD], BF16, kind="ExternalInput").ap()
    bdec_d = nc.dram_tensor("bdec", [FD, 1], F32, kind="ExternalInput").ap()
    y_d = nc.dram_tensor("y", [n_ar + 1, F, BL], BF16, kind="ExternalOutput").ap()

    from contextlib import ExitStack
    with tile.TileContext(nc) as tc, ExitStack() as ctx:
        wpool = ctx.enter_context(tc.tile_pool(name="w", bufs=1))
        spool = ctx.enter_context(tc.tile_pool(name="s", bufs=2))
        hpool = ctx.enter_context(tc.tile_pool(name="h", bufs=2))
        dpool = ctx.enter_context(tc.tile_pool(name="d", bufs=2))
        gpool = ctx.enter_context(tc.tile_pool(name="g", bufs=1, space="PSUM"))

        # ---- resident weights + input ----
        # DMA order matters: everything step 0 touches (x head, m0, b1,
        # ones) loads FIRST; the three 2MB weight matrices follow in
        # first-use order; the 2.7MB x tail (not needed until step 8)
        # goes last. This cut a measured ~25us lead-in stall where
        # bias1(0)/wih1(0) sat behind the bulk transfers.
        x_sb = wpool.tile([KX, n_tf, BL], BF16)
        x_head = min(8, n_tf)
        nc.sync.dma_start(x_sb[:, 0:1, :], x_d[:, 0:1, :])
        m0_sb = wpool.tile([KX, G], BF16)
        nc.sync.dma_start(m0_sb[:], m0_d[:])
        nc.sync.dma_start(x_sb[:, 1:x_head, :], x_d[:, 1:x_head, :])
        b1r_sb = wpool.tile([128, 128], BF16)
        nc.scalar.dma_start(b1r_sb[:], b1r_d[:])
        selr_sb = wpool.tile([128, 512], BF16)
        nc.scalar.dma_start(selr_sb[:], selr_d[:])
        # Warm the ACT spline-table set (sigmoid_and_others covers both
        # Sigmoid and Tanh) during the otherwise-idle preamble window so
        # the ~2.7us ACT_TABLE_LOAD isn't lazily inserted in front of the
        # first real activation at ~13us.
        act_warm = wpool.tile([1, 8], F32)
        nc.vector.memset(act_warm[:], 0.0)
        nc.scalar.activation(act_warm[:], act_warm[:], AF.Sigmoid)
        # The big weight matrices arrive as per-k-chunk DMAs on the SYNC
        # hardware queue in first-use order: wih1 (step 0 mid), whh0
        # (step 0 tail), whh1 (step 1); emit_rec walks k-outer within
        # each half, matching this arrival order. All bulk data stays on
        # sync: DMA triggers on the scalar/Act queue measurably BLOCK the
        # ACT engine's instruction FIFO until the transfer completes
        # (sig_fi(0) stalled 9us when weights rode that queue), and
        # gpsimd's software DGE is ~3x slower to deliver. Only the two
        # tiny bias/selector tiles ride the scalar queue (they land ~5us,
        # well before the first ACT op at ~15us).
        # wih1's upper k-chunks also ride the scalar queue: its 4 trigger
        # instructions all retire by ~10us (each waits only the PREVIOUS
        # transfer's completion), before the first ACT op at ~14us, while
        # offloading 1MB from the bandwidth-bound sync queue exactly when
        # step 0 is racing the weight arrivals.
        wih1_sb = wpool.tile([128, KCH, G], BF16)
        whh0_sb = wpool.tile([128, KCH, G], BF16)
        whh1_sb = wpool.tile([128, KCH, G], BF16)
        for k in range(KCH):
            eng = nc.sync if k < 2 else nc.scalar
            eng.dma_start(wih1_sb[:, k], wih1_d[:, k])
        for w_sb, w_d in ((whh0_sb, whh0_d), (whh1_sb, whh1_d)):
            for k in range(KCH):
                nc.sync.dma_start(w_sb[:, k], w_d[:, k])
        wdec_sb = wpool.tile([128, KCH, FD], BF16)
        nc.sync.dma_start(wdec_sb[:], wdec_d[:])
        bdec_sb = wpool.tile([FD, 1], F32)
        nc.sync.dma_start(bdec_sb[:], bdec_d[:])
        if x_head < n_tf:
            nc.sync.dma_start(x_sb[:, x_head:, :], x_d[:, x_head:, :])

        def halves(ga, gb, m):
            return (ga if m < 8 else gb)[:, m % 8, :]

        # start=True lazily zeroes the WHOLE 2KB PSUM bank (4 of our 512B
        # chunk regions), so only the first write per bank may set it; the
        # sibling regions still see the bank's pending-zero and overwrite.
        def emit_xa(ga, gb, rhs, only=False, accum=False):
            # input-side gate contribution. `only`=True closes the group
            # (t=0 has no recurrent term). `accum`=True: the recurrent
            # matmuls already wrote (and zeroed) the banks, so accumulate
            # on top and close each chunk's group.
            for m in range(NCH):
                nc.tensor.matmul(halves(ga, gb, m),
                                 m0_sb[:, m * 128:(m + 1) * 128], rhs,
                                 start=(not accum) and (m % 4 == 0),
                                 stop=only or accum,
                                 skip_group_check=True)

        def emit_bias1(ga, gb, banks=(0, 1, 2, 3)):
            # One K=4 selector matmul per PSUM bank, row-TILED into the
            # four 32-row groups of the PE array so consecutive banks run
            # concurrently (span ~= one N=512 matmul, not four).
            # out[p, c*128+b] = sum_k b1r[32j+k, p] * selr[32j+k, c*128+b]
            #                 = b1[(4j+c)*128 + p].
            for j in banks:
                gt = ga if j < 2 else gb
                out = gt[:, (j % 2) * 4:(j % 2) * 4 + 4, :]
                nc.tensor.matmul(out, b1r_sb[32 * j:32 * j + 4, :],
                                 selr_sb[32 * j:32 * j + 4, :],
                                 start=True, stop=False,
                                 skip_group_check=True,
                                 tile_position=(32 * j, 0))

        def emit_rec(w_sb, h, ga, gb, last, zero_start=False):
            # k-outer within each 8-chunk half: the first 8 matmuls need
            # only h chunk 0, the next 8 only chunk 1, ... so PE never
            # waits for the cell's staggered per-chunk h production (the
            # DVE emits chunks ~210ns apart; m-outer would need all four
            # within 230ns). The PSUM completion position of each gate
            # half is unchanged (still matmul #32 / #64), so the
            # downstream activation timing is identical.
            for half in range(2):
                ms = range(half * 8, half * 8 + 8)
                for k in range(KCH):
                    for m in ms:
                        nc.tensor.matmul(halves(ga, gb, m),
                                         w_sb[:, k, m * 128:(m + 1) * 128],
                                         h[:, k, :],
                                         start=(zero_start and k == 0
                                                and m % 4 == 0),
                                         stop=(last and k == KCH - 1),
                                         skip_group_check=True)

        def cell(ga, gb, c_prev, l):
            # The serial chain gates -> h bounds how early the next step's
            # h-consuming matmuls can run (the PE covers it with ~3.6us of
            # h-independent work in AR steps). tanh_g / ig / add / tanh_c
            # are split into 2-chunk halves so h chunk 0 emerges ~0.7us
            # sooner; the halves pipeline across the ACT/DVE queues.
            sig_fi = spool.tile([128, 8, BL], F32, tag=f"sfi{l}")
            nc.scalar.activation(sig_fi[:], ga[:], AF.Sigmoid)
            tanh_g = spool.tile([128, KCH, BL], F32, tag=f"tg{l}")
            nc.scalar.activation(tanh_g[:, 0:2, :], gb[:, 4:6, :], AF.Tanh)
            nc.scalar.activation(tanh_g[:, 2:4, :], gb[:, 6:8, :], AF.Tanh)
            sig_o = spool.tile([128, KCH, BL], F32, tag=f"so{l}")
            nc.scalar.activation(sig_o[:], gb[:, 0:4, :], AF.Sigmoid)
            # fc first on the DVE queue: its input sig_fi is ready one Act
            # op earlier than ig's tanh_g, shortening the serial cell chain.
            c_new = hpool.tile([128, KCH, BL], F32, tag=f"c{l}")
            ig = spool.tile([128, KCH, BL], F32, tag=f"ig{l}")
            tanh_c = spool.tile([128, KCH, BL], F32, tag=f"tc{l}")
            if c_prev is None:
                for a, b in ((0, 2), (2, 4)):
                    nc.vector.tensor_mul(ig[:, a:b, :],
                                         sig_fi[:, 4 + a:4 + b, :],
                                         tanh_g[:, a:b, :])
                    nc.vector.tensor_copy(c_new[:, a:b, :], ig[:, a:b, :])
                    nc.scalar.activation(tanh_c[:, a:b, :], c_new[:, a:b, :],
                                         AF.Tanh)
            else:
                fc = spool.tile([128, KCH, BL], F32, tag=f"fc{l}")
                nc.vector.tensor_mul(fc[:], sig_fi[:, 0:4, :], c_prev[:])
                for a, b in ((0, 2), (2, 4)):
                    nc.vector.tensor_mul(ig[:, a:b, :],
                                         sig_fi[:, 4 + a:4 + b, :],
                                         tanh_g[:, a:b, :])
                    nc.vector.tensor_add(c_new[:, a:b, :], fc[:, a:b, :],
                                         ig[:, a:b, :])
                    nc.scalar.activation(tanh_c[:, a:b, :], c_new[:, a:b, :],
                                         AF.Tanh)
            h_new = hpool.tile([128, KCH, BL], BF16, tag=f"h{l}")
            for k in range(KCH):
                nc.vector.tensor_mul(h_new[:, k, :], sig_o[:, k, :],
                                     tanh_c[:, k, :])
            return c_new, h_new

        def emit_dec(j, h1):
            # dec_ps aliases the gb1 slot: cell1(t)'s ACT reads of gb1 are
            # long done when PE reaches the decoder. The gb1 bias banks
            # for step t+1 are (re-)zeroed AFTER the dout read.
            dec_ps = gpool.tile([FD, BL], F32, tag="gb1")
            for k in range(KCH):
                nc.tensor.matmul(dec_ps[:], wdec_sb[:, k, :], h1[:, k, :],
                                 start=(k == 0), stop=(k == KCH - 1))
            # Bias-add on DVE, not ACT: DVE is idle here (h1 muls just
            # drained) while ACT's FIFO latency would sit on the critical
            # dec -> dout -> xa feedback path.
            dout = dpool.tile([FD, BL], BF16, tag="dout")
            nc.vector.tensor_scalar_add(dout[:], dec_ps[:], bdec_sb[:])
            nc.sync.dma_start(y_d[j], dout[0:F, :])
            return dout

        n_steps = n_tf + n_ar
        h0 = h1 = c0 = c1 = None
        ga0 = gb0 = ga1 = gb1 = None
        for t in range(n_steps):
            if t == 0:
                ga0 = gpool.tile([128, 8, BL], F32, tag="ga0")
                gb0 = gpool.tile([128, 8, BL], F32, tag="gb0")
                emit_xa(ga0, gb0, x_sb[:, 0, :], only=True)
            c0, h0 = cell(ga0, gb0, c0, 0)
            if t == 0:
                ga1 = gpool.tile([128, 8, BL], F32, tag="ga1")
                gb1 = gpool.tile([128, 8, BL], F32, tag="gb1")
                emit_bias1(ga1, gb1)
            else:
                emit_rec(whh1_sb, h1, ga1, gb1, last=False)
            emit_rec(wih1_sb, h0, ga1, gb1, last=True)
            c1, h1 = cell(ga1, gb1, c1, 1)

            next_is_ar = n_tf <= t + 1 < n_steps
            if next_is_ar:
                # Fill the wait for h1(t) with h1-independent work for
                # step t+1: the ga1 bias banks and ALL of whh0(t+1)
                # (~4.5us of PE work vs the ~4us cell1 latency).
                ga1n = gpool.tile([128, 8, BL], F32, tag="ga1")
                emit_bias1(ga1n, None, banks=(0, 1))
                ga0 = gpool.tile([128, 8, BL], F32, tag="ga0")
                gb0 = gpool.tile([128, 8, BL], F32, tag="gb0")
                emit_rec(whh0_sb, h0, ga0, gb0, last=False, zero_start=True)
                dout = emit_dec(t - (n_tf - 1), h1)
                # xa BEFORE the gb1 bias banks: both wait on dout, but xa
                # is on the critical path to cell0(t+1) -> h0 -> wih1(t+1)
                # while bias23 only needs to precede whh1(t+1).
                emit_xa(ga0, gb0, dout[0:KX, :], accum=True)
                gb1n = gpool.tile([128, 8, BL], F32, tag="gb1")
                emit_bias1(None, gb1n, banks=(2, 3))
                ga1, gb1 = ga1n, gb1n
            elif t + 1 < n_steps:
                # TF next step: x is SBUF-resident, no feedback hazard.
                ga0 = gpool.tile([128, 8, BL], F32, tag="ga0")
                gb0 = gpool.tile([128, 8, BL], F32, tag="gb0")
                emit_xa(ga0, gb0, x_sb[:, t + 1, :])
                emit_rec(whh0_sb, h0, ga0, gb0, last=True)
                ga1 = gpool.tile([128, 8, BL], F32, tag="ga1")
                gb1 = gpool.tile([128, 8, BL], F32, tag="gb1")
                emit_bias1(ga1, gb1)
            else:
                emit_dec(t - (n_tf - 1), h1)

    nc.compile()
    return nc


def _get_program(n_tf=S, n_ar=PRED_LEN - 1):
    key = (n_tf, n_ar)
    with _cache_lock:
        if key not in _cache:
            _cache[key] = _build_program(n_tf, n_ar)
        return _cache[key]


def _kmajor(w):
    """[H, N] -> [128, KCH, N]: row h = k*128 + p lands at [p, k, :]."""
    n = w.shape[1]
    return np.ascontiguousarray(
        w.reshape(KCH, 128, n).transpose(1, 0, 2)).astype(BF16NP)


def _prep_weights(W_enc, b_enc, Wih0, Whh0, bih0, bhh0,
                  Wih1, Whh1, bih1, bhh1, W_dec, b_dec):
    perm = _gate_perm()
    f32 = np.float32

    M0 = (Wih0 @ W_enc)[perm]                                   # [G, F]
    b0 = (Wih0 @ b_enc + bih0 + bhh0)[perm]                     # [G]
    m0t = np.concatenate([M0.T, b0[None, :]], axis=0)           # [KX, G]

    b1p = (bih1 + bhh1)[perm]                                   # [G]
    # Row-tiled bias layout: bank j's K=4 selector operands live at
    # partitions 32j..32j+3 (PE row group j).
    b1r = np.zeros((128, 128), f32)
    selr = np.zeros((128, 512), f32)
    for j in range(KCH):
        for k in range(KCH):
            b1r[32 * j + k, :] = b1p[(4 * j + k) * 128:(4 * j + k + 1) * 128]
            selr[32 * j + k, k * 128:(k + 1) * 128] = 1.0

    wdec_aug = np.concatenate(
        [W_dec.T, np.zeros((H, 2), f32)], axis=1)               # [H, FD]
    bdec = np.concatenate([b_dec, np.ones((1,), f32), np.zeros((1,), f32)])

    return {
        "m0t": np.ascontiguousarray(m0t).astype(BF16NP),
        "whh0t": _kmajor(np.ascontiguousarray(Whh0[perm].T)),
        "wih1t": _kmajor(np.ascontiguousarray(Wih1[perm].T)),
        "whh1t": _kmajor(np.ascontiguousarray(Whh1[perm].T)),
        "b1r": b1r.astype(BF16NP),
        "selr": selr.astype(BF16NP),
        "wdect": _kmajor(wdec_aug),
        "bdec": np.ascontiguousarray(bdec[:, None], f32),
    }


def _make_in_maps(x, weights, _n_tf=S):
    in_maps = []
    for c in range(NCORES):
        xs = x[c * BL:(c + 1) * BL, :_n_tf, :]                # [BL, n_tf, F]
        xT = xs.transpose(2, 1, 0)                            # [F, n_tf, BL]
        xa = np.concatenate(
            [xT, np.ones((1, _n_tf, BL), np.float32)], axis=0)  # [KX, n_tf, BL]
        in_maps.append(
            {"xT": np.ascontiguousarray(xa).astype(BF16NP), **weights})
    return in_maps


def kernel(x, W_enc, b_enc, Wih0, Whh0, bih0, bhh0,
           Wih1, Whh1, bih1, bhh1, W_dec, b_dec, _n_tf=S, _n_ar=PRED_LEN - 1):
    from concourse.bass_utils import run_bass_kernel_spmd

    x = np.asarray(x, np.float32)
    weights = _prep_weights(
        np.asarray(W_enc, np.float32), np.asarray(b_enc, np.float32),
        np.asarray(Wih0, np.float32), np.asarray(Whh0, np.float32),
        np.asarray(bih0, np.float32), np.asarray(bhh0, np.float32),
        np.asarray(Wih1, np.float32), np.asarray(Whh1, np.float32),
        np.asarray(bih1, np.float32), np.asarray(bhh1, np.float32),
        np.asarray(W_dec, np.float32), np.asarray(b_dec, np.float32))

    nc = _get_program(_n_tf, _n_ar)
    in_maps = _make_in_maps(x, weights, _n_tf)
    res = run_bass_kernel_spmd(nc, in_maps, core_ids=list(range(NCORES)))

    out = np.empty((B, _n_ar + 1, F), np.float32)
    for c in range(NCORES):
        y = np.asarray(res.results[c]["y"], dtype=np.float32)  # [n_out, F, BL]
        out[c * BL:(c + 1) * BL] = y.transpose(2, 0, 1)
    return out
